# revision 7
# baseline (speedup 1.0000x reference)
import sys, os
sys.path.insert(0, '/opt/trn_rl_repo')
import numpy as np
import ml_dtypes

import concourse.bacc as bacc
import concourse.mybir as mybir
from concourse import tile
from concourse.bass_utils import run_bass_kernel_spmd

_orig_get_act_tables = bacc.get_activation_tables
def _pinned_act_tables(arch):
    t = _orig_get_act_tables(arch)
    mine = {mybir.ActivationFunctionType.Exp, mybir.ActivationFunctionType.Ln,
            mybir.ActivationFunctionType.Relu, mybir.ActivationFunctionType.Identity,
            mybir.ActivationFunctionType.Copy}
    out = {}
    for name, fns in t.items():
        if name == "natural_log_exp_and_others":
            out[name] = fns
        else:
            out[name] = fns - mine
    return out
bacc.get_activation_tables = _pinned_act_tables

F32 = mybir.dt.float32
BF16 = mybir.dt.bfloat16
OP = mybir.AluOpType
AF = mybir.ActivationFunctionType
AX = mybir.AxisListType

H = 4
DH = 32
D = 128
N_CORES = 8
INV_SQRT_DH = float(1.0 / np.sqrt(32.0))
BF = ml_dtypes.bfloat16
LAST_RESULT = None


def _build_program(Cs, has_kvbias):
    """dst-major edge layout: blocks of 128 degree-sorted nodes; column j of a
    block holds the j-th edge of every node.  Padding columns contribute
    exp(0)=1 to the softmax denominator, which is subtracted exactly via a
    rank-1 matmul of the per-node pad count.  Aggregation over columns is a
    PSUM-accumulated identity matmul."""
    STAGE = int(os.environ.get("K_STAGE", "9"))
    WV_DVE = bool(os.environ.get("K_WV_DVE"))
    B = len(Cs)
    TOTC = int(sum(Cs))
    Cmax = int(max(Cs))
    NB = B * 128
    colbase = np.concatenate([[0], np.cumsum(Cs)]).astype(int)

    nc = bacc.Bacc(None, target_bir_lowering=False, debug=False)

    ls_in = nc.declare_dram_parameter("ls", [128, TOTC * 128], BF16, isOutput=False)
    ef_in = nc.declare_dram_parameter("ef", [128, TOTC * 128], BF16, isOutput=False)
    q_in = nc.declare_dram_parameter("qrm", [NB, 128], BF16, isOutput=False)
    rt_in = nc.declare_dram_parameter("rt", [128, NB], BF16, isOutput=False)
    pad_in = nc.declare_dram_parameter("padT", [1, NB], BF16, isOutput=False)
    wkv_in = nc.declare_dram_parameter("wkv", [D, 2 * D], BF16, isOutput=False)
    wee_in = nc.declare_dram_parameter("wee", [D, 2 * D], BF16, isOutput=False)
    wsk_in = nc.declare_dram_parameter("wsk", [D, D + 1], BF16, isOutput=False)
    w1a_in = nc.declare_dram_parameter("w1a", [D, D], BF16, isOutput=False)
    w1b_in = nc.declare_dram_parameter("w1b", [D, D], BF16, isOutput=False)
    w2_in = nc.declare_dram_parameter("w2", [D, D], BF16, isOutput=False)
    idb_in = nc.declare_dram_parameter("identB", [128, 128], BF16, isOutput=False)
    idf_in = nc.declare_dram_parameter("identF", [128, 128], F32, isOutput=False)
    ones1_in = nc.declare_dram_parameter("ones1", [1, 128], BF16, isOutput=False)
    neg4_in = nc.declare_dram_parameter("neg4", [1, 4], BF16, isOutput=False)
    eps4_in = nc.declare_dram_parameter("eps4", [1, 4], BF16, isOutput=False)
    bskr_in = nc.declare_dram_parameter("bskr", [1, D + 1], BF16, isOutput=False)
    bkv_in = nc.declare_dram_parameter("bkvr", [1, 2 * D], BF16, isOutput=False)
    uT_in = nc.declare_dram_parameter("uT", [128, D], F32, isOutput=False)
    b1c_in = nc.declare_dram_parameter("b1c", [128, 1], F32, isOutput=False)
    b2c_in = nc.declare_dram_parameter("b2c", [128, 1], F32, isOutput=False)
    out_p = nc.declare_dram_parameter("out", [128, NB], BF16, isOutput=True)

    with tile.TileContext(nc) as tc:
        with (
            tc.tile_pool(name="const", bufs=1) as cpool,
            tc.tile_pool(name="stream", bufs=2) as spool,
            tc.tile_pool(name="kvsb", bufs=2) as kpool,
            tc.tile_pool(name="wva", bufs=2) as wpool,
            tc.tile_pool(name="prod", bufs=2) as ppool,
            tc.tile_pool(name="alf", bufs=2) as apool,
            tc.tile_pool(name="node", bufs=2) as npool,
            tc.tile_pool(name="ps_kv", bufs=2, space="PSUM") as ps_kv,
            tc.tile_pool(name="ps_agg", bufs=2, space="PSUM") as ps_agg,
            tc.tile_pool(name="ps_node", bufs=2, space="PSUM") as ps_node,
        ):
            def ctile(shape, dt, src, tag):
                t = cpool.tile(shape, dt, tag=tag)
                nc.sync.dma_start(t[:], src[:])
                return t
            wkv = ctile([D, 2 * D], BF16, wkv_in, "c_wkv")
            wee = ctile([D, 2 * D], BF16, wee_in, "c_wee")
            wsk = ctile([D, D + 1], BF16, wsk_in, "c_wsk")
            w1a = ctile([D, D], BF16, w1a_in, "c_w1a")
            w1b = ctile([D, D], BF16, w1b_in, "c_w1b")
            w2 = ctile([D, D], BF16, w2_in, "c_w2")
            identB = ctile([128, 128], BF16, idb_in, "c_idb")
            identF = ctile([128, 128], F32, idf_in, "c_idf")
            ones1 = ctile([1, 128], BF16, ones1_in, "c_on")
            neg4 = ctile([1, 4], BF16, neg4_in, "c_n4")
            eps4 = ctile([1, 4], BF16, eps4_in, "c_e4")
            bskr = ctile([1, D + 1], BF16, bskr_in, "c_bskr")
            bkvr = ctile([1, 2 * D], BF16, bkv_in, "c_bkvr")
            uT = ctile([128, D], F32, uT_in, "c_uT")
            b1c = ctile([128, 1], F32, b1c_in, "c_b1")
            b2c = ctile([128, 1], F32, b2c_in, "c_b2")
            padT = ctile([1, NB], BF16, pad_in, "c_pad")
            rt = cpool.tile([128, NB], BF16, tag="c_rt")
            nc.sync.dma_start(rt[:], rt_in[:])
            eps1 = cpool.tile([128, 1], F32, tag="c_eps")
            nc.gpsimd.memset(eps1[:], 1e-5)

            for t in range(B // 2):
                xrp = ps_node.tile([128, 2, 256], F32, tag="nps")
                aggp = ps_agg.tile([128, 2, 256], F32, tag="agg")
                attn = npool.tile([128, 2, 128], BF16, tag="at")
                scr = npool.tile([128, 2, 128], F32, tag="scr")
                st = npool.tile([128, 2, 8], F32, tag="st")
                bst = npool.tile([128, 2, 6], F32, tag="bst")
                rc = npool.tile([128, 2, 4], F32, tag="rc")
                d1 = npool.tile([128, 2, 128], BF16, tag="d1")

                for i in (0, 1):
                    s = 2 * t + i
                    C = int(Cs[s])
                    base = int(colbase[s])
                    ls_b = spool.tile([128, Cmax * 128], BF16, tag="ls")
                    ef_b = spool.tile([128, Cmax * 128], BF16, tag="ef")
                    q_b = spool.tile([128, 128], BF16, tag="q")
                    nc.sync.dma_start(ls_b[:, 0:C * 128], ls_in[:, base * 128:(base + C) * 128])
                    nc.sync.dma_start(ef_b[:, 0:C * 128], ef_in[:, base * 128:(base + C) * 128])
                    nc.sync.dma_start(q_b[:], q_in[128 * s:128 * (s + 1), :])
                    kv_sb = kpool.tile([128, Cmax, 256], BF16, tag="kv")
                    wv_aug = wpool.tile([128, Cmax, 132], BF16, tag="wv")
                    alpha = apool.tile([128, Cmax * 4], F32, tag="al")

                    n_g = (C + 3) // 4
                    for g in range(n_g):
                        c0 = 4 * g
                        gc = min(4, C - c0)
                        kvp = ps_kv.tile([128, 4, 256], F32, tag="kvp")
                        for ci in range(gc):
                            c = c0 + ci
                            sl = slice(128 * c, 128 * (c + 1))
                            nc.tensor.matmul(kvp[:, ci, :], ls_b[:, sl], wkv[:], start=True, stop=False)
                            nc.tensor.matmul(kvp[:, ci, :], ef_b[:, sl], wee[:],
                                             start=False, stop=(not has_kvbias))
                            if has_kvbias:
                                nc.tensor.matmul(kvp[:, ci, :], ones1[:], bkvr[:], start=False, stop=True)
                        nc.scalar.copy(kv_sb[:, c0:c0 + gc, :], kvp[:, 0:gc, :])
                    prod = ppool.tile([128, Cmax, 128], BF16, tag="pr")
                    nc.vector.tensor_tensor(
                        prod[:, 0:C, :], kv_sb[:, 0:C, 0:128],
                        q_b[:].unsqueeze(1).broadcast_to([128, C, 128]), OP.mult)
                    nc.vector.tensor_reduce(
                        alpha[:, 0:C * 4],
                        prod[:, 0:C, :].rearrange("p a (h d) -> p (a h) d", d=DH),
                        AX.X, OP.add)
                    nc.scalar.activation(wv_aug[:, 0:C, 128:132], alpha[:, 0:C * 4],
                                         AF.Exp, scale=INV_SQRT_DH)
                    wv_eng = nc.vector if WV_DVE else nc.gpsimd
                    wv_eng.tensor_tensor(
                        wv_aug[:, 0:C, 0:128].rearrange("p a (h d) -> p a h d", d=DH),
                        kv_sb[:, 0:C, 128:256].rearrange("p a (h d) -> p a h d", d=DH),
                        wv_aug[:, 0:C, 128:132].unsqueeze(3).broadcast_to([128, C, H, DH]),
                        OP.mult)
                    for c in range(C):
                        nc.tensor.matmul(aggp[:, i, 0:132], identB[:], wv_aug[:, c, :],
                                         start=(c == 0), stop=False)
                    nc.tensor.matmul(aggp[:, i, 128:132], padT[:, 128 * s:128 * (s + 1)], neg4[:],
                                     start=False, stop=False)
                    nc.tensor.matmul(aggp[:, i, 128:132], ones1[:], eps4[:],
                                     start=False, stop=True)
                    nc.tensor.matmul(xrp[:, i, 0:129], rt[:, 128 * s:128 * (s + 1)], wsk[:],
                                     start=True, stop=False)
                    nc.tensor.matmul(xrp[:, i, 0:129], ones1[:], bskr[:], start=False, stop=True)

                # ---- node tail for the pair ----
                nc.vector.reciprocal(rc[:], aggp[:, :, 128:132])
                nc.vector.tensor_tensor(
                    attn[:].rearrange("p a (h d) -> p a h d", d=DH),
                    aggp[:, :, 0:128].rearrange("p a (h d) -> p a h d", d=DH),
                    rc[:].unsqueeze(3).broadcast_to([128, 2, H, DH]), OP.mult)
                if STAGE < 2:
                    outs0 = npool.tile([128, 2, 128], BF16, tag="os")
                    nc.vector.tensor_copy(outs0[:], attn[:])
                    nc.sync.dma_start(out_p[:, 256 * t:256 * (t + 1)],
                                      outs0[:].rearrange("p a b -> p (a b)"))
                    continue
                for i in (0, 1):
                    nc.vector.scalar_tensor_tensor(
                        scr[:, i, :], attn[:, i, :], 1.0, uT[:], OP.mult, OP.mult,
                        accum_out=st[:, i, 0:1])
                nc.vector.tensor_copy(st[:, :, 1:2], xrp[:, :, 128:129])
                nc.gpsimd.tensor_tensor(st[:, :, 2:3], st[:, :, 0:1], st[:, :, 1:2], OP.add)
                nc.scalar.activation(st[:, :, 3:4], st[:, :, 2:3], AF.Exp, scale=-1.0)
                nc.gpsimd.tensor_scalar(st[:, :, 4:5], st[:, :, 3:4], 1.0, None, OP.add)
                nc.vector.reciprocal(st[:, :, 5:6], st[:, :, 4:5])
                nc.vector.tensor_tensor(d1[:], xrp[:, :, 0:128], attn[:], OP.subtract)
                msg = npool.tile([128, 2, 128], BF16, tag="mg")
                for i in (0, 1):
                    nc.vector.scalar_tensor_tensor(
                        msg[:, i, :], d1[:, i, :], st[:, i, 5:6], attn[:, i, :],
                        OP.mult, OP.add)
                if STAGE < 3:
                    nc.sync.dma_start(out_p[:, 256 * t:256 * (t + 1)],
                                      msg[:].rearrange("p a b -> p (a b)"))
                    continue
                for i in (0, 1):
                    nc.vector.bn_stats(bst[:, i, :], msg[:, i, :])
                    nc.vector.bn_aggr(st[:, i, 6:8], bst[:, i, :])
                nc.scalar.activation(st[:, :, 2:3], st[:, :, 7:8], AF.Ln, bias=eps1[:, :])
                nc.scalar.activation(st[:, :, 3:4], st[:, :, 2:3], AF.Exp, scale=-0.5)
                cen = npool.tile([128, 2, 128], F32, tag="cn")
                for i in (0, 1):
                    nc.vector.scalar_tensor_tensor(
                        cen[:, i, :], msg[:, i, :], st[:, i, 6:7],
                        st[:, i, 3:4].broadcast_to([128, 128]), OP.subtract, OP.mult)
                if STAGE < 4:
                    outs1 = npool.tile([128, 2, 128], BF16, tag="os")
                    nc.vector.tensor_copy(outs1[:], cen[:])
                    nc.sync.dma_start(out_p[:, 256 * t:256 * (t + 1)],
                                      outs1[:].rearrange("p a b -> p (a b)"))
                    continue
                tpp = ps_node.tile([128, 2, 256], F32, tag="nps")
                for i in (0, 1):
                    nc.tensor.transpose(tpp[:, i, 0:128], cen[:, i, :], identF[:])
                msgT = npool.tile([128, 2, 128], BF16, tag="mt")
                nc.vector.tensor_copy(msgT[:], tpp[:, :, 0:128])
                h1p = ps_node.tile([128, 2, 256], F32, tag="nps")
                for i in (0, 1):
                    s = 2 * t + i
                    nc.tensor.matmul(h1p[:, i, 0:128], w1a[:], msgT[:, i, :], start=True, stop=False)
                    nc.tensor.matmul(h1p[:, i, 0:128], w1b[:], rt[:, 128 * s:128 * (s + 1)],
                                     start=False, stop=True)
                h1s = npool.tile([128, 2, 128], BF16, tag="h1")
                nc.scalar.activation(h1s[:], h1p[:, :, 0:128], AF.Relu, bias=b1c[:, :])
                outp = ps_node.tile([128, 2, 256], F32, tag="nps")
                for i in (0, 1):
                    nc.tensor.matmul(outp[:, i, 0:128], w2[:], h1s[:, i, :], start=True, stop=True)
                outs = npool.tile([128, 2, 128], BF16, tag="os")
                nc.scalar.activation(outs[:], outp[:, :, 0:128], AF.Identity, bias=b2c[:, :])
                nc.sync.dma_start(out_p[:, 256 * t:256 * (t + 1)],
                                  outs[:].rearrange("p a b -> p (a b)"))

    nc.finalize()
    return nc


def kernel(left_features, edge_indices, edge_features, right_features,
           Wq, bq, Wk, bk, Wv, bv, We, Wskip, bskip, Wbeta,
           ln_g, ln_b, W1, b1, W2, b2):
    left_features = np.asarray(left_features, np.float32)
    edge_features = np.asarray(edge_features, np.float32)
    right_features = np.asarray(right_features, np.float32)
    ei = np.asarray(edge_indices).astype(np.int64)
    src, dst = ei[0], ei[1]
    E = src.shape[0]
    NR = right_features.shape[0]

    Wq = np.asarray(Wq, np.float32); Wk = np.asarray(Wk, np.float32)
    Wv = np.asarray(Wv, np.float32); We = np.asarray(We, np.float32)
    Wskip = np.asarray(Wskip, np.float32)
    Wbeta = np.asarray(Wbeta, np.float32).reshape(3 * D)
    W1 = np.asarray(W1, np.float32); W2 = np.asarray(W2, np.float32)
    bq = np.asarray(bq, np.float32); bk = np.asarray(bk, np.float32)
    bv = np.asarray(bv, np.float32); bskip = np.asarray(bskip, np.float32)
    b1 = np.asarray(b1, np.float32); b2 = np.asarray(b2, np.float32)
    ln_g = np.asarray(ln_g, np.float32); ln_b = np.asarray(ln_b, np.float32)
    u_vec = Wbeta[0:D] + Wbeta[2 * D:3 * D]
    w_vec = Wbeta[D:2 * D] - Wbeta[2 * D:3 * D]

    # ---- node ordering: degree-sorted blocks of 128 ----
    deg = np.bincount(dst, minlength=NR)
    order = np.argsort(-deg, kind='stable')
    rank = np.empty(NR, np.int64)
    rank[order] = np.arange(NR)
    B = int(np.ceil(NR / (128.0 * N_CORES)))  # slots per core
    if B % 2:
        B += 1  # tail is processed in pairs of slots
    NBLK = B * N_CORES
    NPAD = NBLK * 128
    deg_sorted = np.zeros(NPAD, np.int64)
    deg_sorted[:NR] = deg[order]
    Cs = np.maximum(1, deg_sorted[np.arange(B) * 128 * N_CORES]).astype(int)
    colbase = np.concatenate([[0], np.cumsum(Cs)]).astype(int)
    TOTC = int(Cs.sum())
    Cmax = int(Cs.max())

    # ---- per-edge placement ----
    r_dst = rank[dst]
    eo = np.argsort(r_dst, kind='stable')
    rs = r_dst[eo]
    node_starts = np.searchsorted(rs, np.arange(NR))
    j_in_node = np.arange(E) - node_starts[rs]
    blk = rs // 128
    s_of = blk // N_CORES
    core_of = blk % N_CORES
    n128 = rs % 128
    flatcol = (colbase[s_of] + j_in_node) * 128 + n128

    left_bf = left_features.astype(BF)
    ef_bf = edge_features.astype(BF)
    src_eo = src[eo]
    eidx_eo = eo

    # ---- host-side q (part of per-destination gather prep) ----
    q_full = (right_features @ Wq + bq).astype(BF)
    rt_sorted = np.zeros((NPAD, D), BF)
    rt_sorted[:NR] = right_features[order].astype(BF)
    q_sorted = np.zeros((NPAD, D), BF)
    q_sorted[:NR] = q_full[order]

    has_kvbias = bool(np.any(bv))
    nc = _build_program(Cs, has_kvbias)

    identF = np.eye(128, dtype=np.float32)
    identB = identF.astype(BF)
    w1a_s = (ln_g[:, None] * W1[0:D, :]).astype(BF)
    b1c = (b1 + W1[0:D, :].T @ ln_b).astype(np.float32).reshape(128, 1)
    bskr = np.concatenate([bskip, [float(bskip @ w_vec)]]).reshape(1, D + 1).astype(BF)
    bkvr = np.concatenate([np.zeros(D, np.float32), bv]).reshape(1, 2 * D).astype(BF)
    # per-node pad count (Cs[slot] - deg), subtracted from softmax denominator
    slot_of_rank = (np.arange(NPAD) // 128) // N_CORES
    pad_cnt = (Cs[slot_of_rank] - deg_sorted).astype(np.float32)

    in_maps = []
    for core in range(N_CORES):
        m = core_of == core
        cc = flatcol[m]
        ls_c = np.zeros((TOTC * 128, D), BF)
        ef_c = np.zeros((TOTC * 128, D), BF)
        ls_c[cc] = left_bf[src_eo[m]]
        ef_c[cc] = ef_bf[eidx_eo[m]]
        # rows for this core: rank r = 128*(N_CORES*s + core) + n
        row_idx = (128 * (N_CORES * np.arange(B)[:, None] + core) +
                   np.arange(128)[None, :]).reshape(-1)
        in_maps.append({
            "ls": ls_c.T.copy(), "ef": ef_c.T.copy(),
            "qrm": q_sorted[row_idx].copy(),
            "rt": rt_sorted[row_idx].T.copy(),
            "padT": pad_cnt[row_idx].reshape(1, B * 128).astype(BF),
            "wkv": np.concatenate([Wk, Wv], 1).astype(BF),
            "wee": np.concatenate([We, We], 1).astype(BF),
            "wsk": np.concatenate([Wskip, (Wskip @ w_vec)[:, None]], 1).astype(BF),
            "w1a": w1a_s, "w1b": W1[D:2 * D, :].astype(BF), "w2": W2.astype(BF),
            "identB": identB, "identF": identF,
            "ones1": np.ones((1, 128), BF),
            "neg4": np.full((1, 4), -1.0, BF),
            "eps4": np.full((1, 4), 1e-16, BF),
            "bskr": bskr, "bkvr": bkvr,
            "uT": np.tile(u_vec.reshape(1, D), (128, 1)),
            "b1c": b1c, "b2c": b2.reshape(128, 1).astype(np.float32),
        })

    trace = bool(os.environ.get("K_TRACE"))
    res = run_bass_kernel_spmd(nc, in_maps, list(range(N_CORES)), trace=trace,
                               tmpdir=os.environ.get("K_TRACE_DIR") or None)
    global LAST_RESULT
    LAST_RESULT = res

    out_full = np.empty((NR, D), np.float32)
    for core in range(N_CORES):
        oc = np.asarray(res.results[core]["out"], dtype=np.float32)  # [128, B*128]
        row_idx = (128 * (N_CORES * np.arange(B)[:, None] + core) +
                   np.arange(128)[None, :]).reshape(-1)
        valid = row_idx < NR
        out_full[order[row_idx[valid]]] = oc.T[valid]
    return out_full


# revision 9
# speedup vs baseline: 1.2124x; 1.2124x over previous
import sys, os
sys.path.insert(0, '/opt/trn_rl_repo')
import numpy as np
import ml_dtypes

import concourse.bacc as bacc
import concourse.mybir as mybir
from concourse import tile
from concourse.bass_utils import run_bass_kernel_spmd

_orig_get_act_tables = bacc.get_activation_tables
def _pinned_act_tables(arch):
    t = _orig_get_act_tables(arch)
    mine = {mybir.ActivationFunctionType.Exp, mybir.ActivationFunctionType.Ln,
            mybir.ActivationFunctionType.Relu, mybir.ActivationFunctionType.Identity,
            mybir.ActivationFunctionType.Copy}
    out = {}
    for name, fns in t.items():
        if name == "natural_log_exp_and_others":
            out[name] = fns
        else:
            out[name] = fns - mine
    return out
bacc.get_activation_tables = _pinned_act_tables

F32 = mybir.dt.float32
BF16 = mybir.dt.bfloat16
OP = mybir.AluOpType
AF = mybir.ActivationFunctionType
AX = mybir.AxisListType

H = 4
DH = 32
D = 128
N_CORES = 8
INV_SQRT_DH = float(1.0 / np.sqrt(32.0))
BF = ml_dtypes.bfloat16
LAST_RESULT = None


def _build_program(Cs, has_kvbias):
    """dst-major edge layout: blocks of 128 degree-sorted nodes; column j of a
    block holds the j-th edge of every node.  Padding columns contribute
    exp(0)=1 to the softmax denominator, which is subtracted exactly via a
    rank-1 matmul of the per-node pad count.  Aggregation over columns is a
    PSUM-accumulated identity matmul."""
    STAGE = int(os.environ.get("K_STAGE", "9"))
    WV_DVE = bool(os.environ.get("K_WV_DVE"))
    B = len(Cs)
    TOTC = int(sum(Cs))
    Cmax = int(max(Cs))
    NB = B * 128
    colbase = np.concatenate([[0], np.cumsum(Cs)]).astype(int)

    nc = bacc.Bacc(None, target_bir_lowering=False, debug=False)

    ls_in = nc.declare_dram_parameter("ls", [128, TOTC * 128], BF16, isOutput=False)
    ef_in = nc.declare_dram_parameter("ef", [128, TOTC * 128], BF16, isOutput=False)
    q_in = nc.declare_dram_parameter("qrm", [NB, 128], BF16, isOutput=False)
    rt_in = nc.declare_dram_parameter("rt", [128, NB], BF16, isOutput=False)
    pad_in = nc.declare_dram_parameter("padT", [1, NB], BF16, isOutput=False)
    wkv_in = nc.declare_dram_parameter("wkv", [D, 2 * D], BF16, isOutput=False)
    wee_in = nc.declare_dram_parameter("wee", [D, 2 * D], BF16, isOutput=False)
    wsk_in = nc.declare_dram_parameter("wsk", [D, D + 1], BF16, isOutput=False)
    w1a_in = nc.declare_dram_parameter("w1a", [D, D], BF16, isOutput=False)
    w1b_in = nc.declare_dram_parameter("w1b", [D, D], BF16, isOutput=False)
    w2_in = nc.declare_dram_parameter("w2", [D, D], BF16, isOutput=False)
    idb_in = nc.declare_dram_parameter("identB", [128, 128], BF16, isOutput=False)
    idf_in = nc.declare_dram_parameter("identF", [128, 128], F32, isOutput=False)
    ones1_in = nc.declare_dram_parameter("ones1", [1, 128], BF16, isOutput=False)
    neg4_in = nc.declare_dram_parameter("neg4", [1, 4], BF16, isOutput=False)
    eps4_in = nc.declare_dram_parameter("eps4", [1, 4], BF16, isOutput=False)
    bskr_in = nc.declare_dram_parameter("bskr", [1, D + 1], BF16, isOutput=False)
    bkv_in = nc.declare_dram_parameter("bkvr", [1, 2 * D], BF16, isOutput=False)
    uT_in = nc.declare_dram_parameter("uT", [128, D], F32, isOutput=False)
    b1c_in = nc.declare_dram_parameter("b1c", [128, 1], F32, isOutput=False)
    b2c_in = nc.declare_dram_parameter("b2c", [128, 1], F32, isOutput=False)
    out_p = nc.declare_dram_parameter("out", [128, NB], BF16, isOutput=True)

    with tile.TileContext(nc) as tc:
        with (
            tc.tile_pool(name="const", bufs=1) as cpool,
            tc.tile_pool(name="stream", bufs=2) as spool,
            tc.tile_pool(name="kvsb", bufs=2) as kpool,
            tc.tile_pool(name="wva", bufs=2) as wpool,
            tc.tile_pool(name="prod", bufs=2) as ppool,
            tc.tile_pool(name="alf", bufs=2) as apool,
            tc.tile_pool(name="node", bufs=2) as npool,
            tc.tile_pool(name="ps_kv", bufs=2, space="PSUM") as ps_kv,
            tc.tile_pool(name="ps_agg", bufs=2, space="PSUM") as ps_agg,
            tc.tile_pool(name="ps_node", bufs=2, space="PSUM") as ps_node,
        ):
            def ctile(shape, dt, src, tag):
                t = cpool.tile(shape, dt, tag=tag)
                nc.sync.dma_start(t[:], src[:])
                return t
            wkv = ctile([D, 2 * D], BF16, wkv_in, "c_wkv")
            wee = ctile([D, 2 * D], BF16, wee_in, "c_wee")
            wsk = ctile([D, D + 1], BF16, wsk_in, "c_wsk")
            w1a = ctile([D, D], BF16, w1a_in, "c_w1a")
            w1b = ctile([D, D], BF16, w1b_in, "c_w1b")
            w2 = ctile([D, D], BF16, w2_in, "c_w2")
            identB = ctile([128, 128], BF16, idb_in, "c_idb")
            identF = ctile([128, 128], F32, idf_in, "c_idf")
            ones1 = ctile([1, 128], BF16, ones1_in, "c_on")
            neg4 = ctile([1, 4], BF16, neg4_in, "c_n4")
            eps4 = ctile([1, 4], BF16, eps4_in, "c_e4")
            bskr = ctile([1, D + 1], BF16, bskr_in, "c_bskr")
            bkvr = ctile([1, 2 * D], BF16, bkv_in, "c_bkvr")
            uT = ctile([128, D], F32, uT_in, "c_uT")
            b1c = ctile([128, 1], F32, b1c_in, "c_b1")
            b2c = ctile([128, 1], F32, b2c_in, "c_b2")
            padT = ctile([1, NB], BF16, pad_in, "c_pad")
            rt = cpool.tile([128, NB], BF16, tag="c_rt")
            nc.sync.dma_start(rt[:], rt_in[:])
            eps1 = cpool.tile([128, 1], F32, tag="c_eps")
            nc.gpsimd.memset(eps1[:], 1e-5)

            for t in range(B // 2):
                xrp = ps_node.tile([128, 2, 256], F32, tag="nps")
                aggp = ps_agg.tile([128, 2, 256], F32, tag="agg")
                attn = npool.tile([128, 2, 128], BF16, tag="at")
                scr = npool.tile([128, 2, 128], F32, tag="scr")
                st = npool.tile([128, 2, 8], F32, tag="st")
                bst = npool.tile([128, 2, 6], F32, tag="bst")
                rc = npool.tile([128, 2, 4], F32, tag="rc")
                d1 = npool.tile([128, 2, 128], BF16, tag="d1")

                for i in (0, 1):
                    s = 2 * t + i
                    C = int(Cs[s])
                    base = int(colbase[s])
                    ls_b = spool.tile([128, Cmax * 128], BF16, tag="ls")
                    ef_b = spool.tile([128, Cmax * 128], BF16, tag="ef")
                    q_b = spool.tile([128, 128], BF16, tag="q")
                    nc.sync.dma_start(ls_b[:, 0:C * 128], ls_in[:, base * 128:(base + C) * 128])
                    nc.sync.dma_start(ef_b[:, 0:C * 128], ef_in[:, base * 128:(base + C) * 128])
                    nc.sync.dma_start(q_b[:], q_in[128 * s:128 * (s + 1), :])
                    kv_sb = kpool.tile([128, Cmax, 256], BF16, tag="kv")
                    wv_aug = wpool.tile([128, Cmax, 132], BF16, tag="wv")
                    alpha = apool.tile([128, Cmax * 4], F32, tag="al")

                    n_g = (C + 3) // 4
                    for g in range(n_g):
                        c0 = 4 * g
                        gc = min(4, C - c0)
                        kvp = ps_kv.tile([128, 4, 256], F32, tag="kvp")
                        for ci in range(gc):
                            c = c0 + ci
                            sl = slice(128 * c, 128 * (c + 1))
                            nc.tensor.matmul(kvp[:, ci, :], ls_b[:, sl], wkv[:], start=True, stop=False)
                            nc.tensor.matmul(kvp[:, ci, :], ef_b[:, sl], wee[:],
                                             start=False, stop=(not has_kvbias))
                            if has_kvbias:
                                nc.tensor.matmul(kvp[:, ci, :], ones1[:], bkvr[:], start=False, stop=True)
                        nc.scalar.copy(kv_sb[:, c0:c0 + gc, :], kvp[:, 0:gc, :])
                        prod = ppool.tile([128, 4, 128], BF16, tag="pr")
                        nc.vector.tensor_tensor(
                            prod[:, 0:gc, :], kv_sb[:, c0:c0 + gc, 0:128],
                            q_b[:].unsqueeze(1).broadcast_to([128, gc, 128]), OP.mult)
                        nc.vector.tensor_reduce(
                            alpha[:, c0 * 4:(c0 + gc) * 4],
                            prod[:, 0:gc, :].rearrange("p a (h d) -> p (a h) d", d=DH),
                            AX.X, OP.add)
                    nc.scalar.activation(wv_aug[:, 0:C, 128:132], alpha[:, 0:C * 4],
                                         AF.Exp, scale=INV_SQRT_DH)
                    wv_eng = nc.vector if WV_DVE else nc.gpsimd
                    for g in range(n_g):
                        c0 = 4 * g
                        gc = min(4, C - c0)
                        wv_eng.tensor_tensor(
                            wv_aug[:, c0:c0 + gc, 0:128].rearrange("p a (h d) -> p a h d", d=DH),
                            kv_sb[:, c0:c0 + gc, 128:256].rearrange("p a (h d) -> p a h d", d=DH),
                            wv_aug[:, c0:c0 + gc, 128:132].unsqueeze(3).broadcast_to([128, gc, H, DH]),
                            OP.mult)
                    for c in range(C):
                        nc.tensor.matmul(aggp[:, i, 0:132], identB[:], wv_aug[:, c, :],
                                         start=(c == 0), stop=False)
                    nc.tensor.matmul(aggp[:, i, 128:132], padT[:, 128 * s:128 * (s + 1)], neg4[:],
                                     start=False, stop=False)
                    nc.tensor.matmul(aggp[:, i, 128:132], ones1[:], eps4[:],
                                     start=False, stop=True)
                    nc.tensor.matmul(xrp[:, i, 0:129], rt[:, 128 * s:128 * (s + 1)], wsk[:],
                                     start=True, stop=False)
                    nc.tensor.matmul(xrp[:, i, 0:129], ones1[:], bskr[:], start=False, stop=True)

                # ---- node tail for the pair ----
                nc.vector.reciprocal(rc[:], aggp[:, :, 128:132])
                nc.vector.tensor_tensor(
                    attn[:].rearrange("p a (h d) -> p a h d", d=DH),
                    aggp[:, :, 0:128].rearrange("p a (h d) -> p a h d", d=DH),
                    rc[:].unsqueeze(3).broadcast_to([128, 2, H, DH]), OP.mult)
                if STAGE < 2:
                    outs0 = npool.tile([128, 2, 128], BF16, tag="os")
                    nc.vector.tensor_copy(outs0[:], attn[:])
                    nc.sync.dma_start(out_p[:, 256 * t:256 * (t + 1)],
                                      outs0[:].rearrange("p a b -> p (a b)"))
                    continue
                for i in (0, 1):
                    nc.vector.scalar_tensor_tensor(
                        scr[:, i, :], attn[:, i, :], 1.0, uT[:], OP.mult, OP.mult,
                        accum_out=st[:, i, 0:1])
                nc.vector.tensor_copy(st[:, :, 1:2], xrp[:, :, 128:129])
                nc.vector.tensor_tensor(st[:, :, 2:3], st[:, :, 0:1], st[:, :, 1:2], OP.add)
                nc.scalar.activation(st[:, :, 3:4], st[:, :, 2:3], AF.Exp, scale=-1.0)
                nc.vector.tensor_scalar(st[:, :, 4:5], st[:, :, 3:4], 1.0, None, OP.add)
                nc.vector.reciprocal(st[:, :, 5:6], st[:, :, 4:5])
                nc.vector.tensor_tensor(d1[:], xrp[:, :, 0:128], attn[:], OP.subtract)
                msg = npool.tile([128, 2, 128], BF16, tag="mg")
                for i in (0, 1):
                    nc.vector.scalar_tensor_tensor(
                        msg[:, i, :], d1[:, i, :], st[:, i, 5:6], attn[:, i, :],
                        OP.mult, OP.add)
                if STAGE < 3:
                    nc.sync.dma_start(out_p[:, 256 * t:256 * (t + 1)],
                                      msg[:].rearrange("p a b -> p (a b)"))
                    continue
                for i in (0, 1):
                    nc.vector.bn_stats(bst[:, i, :], msg[:, i, :])
                    nc.vector.bn_aggr(st[:, i, 6:8], bst[:, i, :])
                nc.scalar.activation(st[:, :, 2:3], st[:, :, 7:8], AF.Ln, bias=eps1[:, :])
                nc.scalar.activation(st[:, :, 3:4], st[:, :, 2:3], AF.Exp, scale=-0.5)
                cen = npool.tile([128, 2, 128], F32, tag="cn")
                for i in (0, 1):
                    nc.vector.scalar_tensor_tensor(
                        cen[:, i, :], msg[:, i, :], st[:, i, 6:7],
                        st[:, i, 3:4].broadcast_to([128, 128]), OP.subtract, OP.mult)
                if STAGE < 4:
                    outs1 = npool.tile([128, 2, 128], BF16, tag="os")
                    nc.vector.tensor_copy(outs1[:], cen[:])
                    nc.sync.dma_start(out_p[:, 256 * t:256 * (t + 1)],
                                      outs1[:].rearrange("p a b -> p (a b)"))
                    continue
                tpp = ps_node.tile([128, 2, 256], F32, tag="nps")
                for i in (0, 1):
                    nc.tensor.transpose(tpp[:, i, 0:128], cen[:, i, :], identF[:])
                msgT = npool.tile([128, 2, 128], BF16, tag="mt")
                nc.vector.tensor_copy(msgT[:], tpp[:, :, 0:128])
                h1p = ps_node.tile([128, 2, 256], F32, tag="nps")
                for i in (0, 1):
                    s = 2 * t + i
                    nc.tensor.matmul(h1p[:, i, 0:128], w1a[:], msgT[:, i, :], start=True, stop=False)
                    nc.tensor.matmul(h1p[:, i, 0:128], w1b[:], rt[:, 128 * s:128 * (s + 1)],
                                     start=False, stop=True)
                h1s = npool.tile([128, 2, 128], BF16, tag="h1")
                nc.scalar.activation(h1s[:], h1p[:, :, 0:128], AF.Relu, bias=b1c[:, :])
                outp = ps_node.tile([128, 2, 256], F32, tag="nps")
                for i in (0, 1):
                    nc.tensor.matmul(outp[:, i, 0:128], w2[:], h1s[:, i, :], start=True, stop=True)
                outs = npool.tile([128, 2, 128], BF16, tag="os")
                nc.scalar.activation(outs[:], outp[:, :, 0:128], AF.Identity, bias=b2c[:, :])
                nc.sync.dma_start(out_p[:, 256 * t:256 * (t + 1)],
                                  outs[:].rearrange("p a b -> p (a b)"))

    nc.finalize()
    return nc


def kernel(left_features, edge_indices, edge_features, right_features,
           Wq, bq, Wk, bk, Wv, bv, We, Wskip, bskip, Wbeta,
           ln_g, ln_b, W1, b1, W2, b2):
    left_features = np.asarray(left_features, np.float32)
    edge_features = np.asarray(edge_features, np.float32)
    right_features = np.asarray(right_features, np.float32)
    ei = np.asarray(edge_indices).astype(np.int64)
    src, dst = ei[0], ei[1]
    E = src.shape[0]
    NR = right_features.shape[0]

    Wq = np.asarray(Wq, np.float32); Wk = np.asarray(Wk, np.float32)
    Wv = np.asarray(Wv, np.float32); We = np.asarray(We, np.float32)
    Wskip = np.asarray(Wskip, np.float32)
    Wbeta = np.asarray(Wbeta, np.float32).reshape(3 * D)
    W1 = np.asarray(W1, np.float32); W2 = np.asarray(W2, np.float32)
    bq = np.asarray(bq, np.float32); bk = np.asarray(bk, np.float32)
    bv = np.asarray(bv, np.float32); bskip = np.asarray(bskip, np.float32)
    b1 = np.asarray(b1, np.float32); b2 = np.asarray(b2, np.float32)
    ln_g = np.asarray(ln_g, np.float32); ln_b = np.asarray(ln_b, np.float32)
    u_vec = Wbeta[0:D] + Wbeta[2 * D:3 * D]
    w_vec = Wbeta[D:2 * D] - Wbeta[2 * D:3 * D]

    # ---- node ordering: degree-sorted blocks of 128 ----
    deg = np.bincount(dst, minlength=NR)
    order = np.argsort(-deg, kind='stable')
    rank = np.empty(NR, np.int64)
    rank[order] = np.arange(NR)
    B = int(np.ceil(NR / (128.0 * N_CORES)))  # slots per core
    if B % 2:
        B += 1  # tail is processed in pairs of slots
    NBLK = B * N_CORES
    NPAD = NBLK * 128
    deg_sorted = np.zeros(NPAD, np.int64)
    deg_sorted[:NR] = deg[order]
    Cs = np.maximum(1, deg_sorted[np.arange(B) * 128 * N_CORES]).astype(int)
    colbase = np.concatenate([[0], np.cumsum(Cs)]).astype(int)
    TOTC = int(Cs.sum())
    Cmax = int(Cs.max())

    # ---- per-edge placement ----
    r_dst = rank[dst]
    eo = np.argsort(r_dst, kind='stable')
    rs = r_dst[eo]
    node_starts = np.searchsorted(rs, np.arange(NR))
    j_in_node = np.arange(E) - node_starts[rs]
    blk = rs // 128
    s_of = blk // N_CORES
    core_of = blk % N_CORES
    n128 = rs % 128
    flatcol = (colbase[s_of] + j_in_node) * 128 + n128

    left_bf = left_features.astype(BF)
    ef_bf = edge_features.astype(BF)
    src_eo = src[eo]
    eidx_eo = eo

    # ---- host-side q (part of per-destination gather prep) ----
    q_full = (right_features @ Wq + bq).astype(BF)
    rt_sorted = np.zeros((NPAD, D), BF)
    rt_sorted[:NR] = right_features[order].astype(BF)
    q_sorted = np.zeros((NPAD, D), BF)
    q_sorted[:NR] = q_full[order]

    has_kvbias = bool(np.any(bv))
    nc = _build_program(Cs, has_kvbias)

    identF = np.eye(128, dtype=np.float32)
    identB = identF.astype(BF)
    w1a_s = (ln_g[:, None] * W1[0:D, :]).astype(BF)
    b1c = (b1 + W1[0:D, :].T @ ln_b).astype(np.float32).reshape(128, 1)
    bskr = np.concatenate([bskip, [float(bskip @ w_vec)]]).reshape(1, D + 1).astype(BF)
    bkvr = np.concatenate([np.zeros(D, np.float32), bv]).reshape(1, 2 * D).astype(BF)
    # per-node pad count (Cs[slot] - deg), subtracted from softmax denominator
    slot_of_rank = (np.arange(NPAD) // 128) // N_CORES
    pad_cnt = (Cs[slot_of_rank] - deg_sorted).astype(np.float32)

    in_maps = []
    for core in range(N_CORES):
        m = core_of == core
        cc = flatcol[m]
        ls_c = np.zeros((TOTC * 128, D), BF)
        ef_c = np.zeros((TOTC * 128, D), BF)
        ls_c[cc] = left_bf[src_eo[m]]
        ef_c[cc] = ef_bf[eidx_eo[m]]
        # rows for this core: rank r = 128*(N_CORES*s + core) + n
        row_idx = (128 * (N_CORES * np.arange(B)[:, None] + core) +
                   np.arange(128)[None, :]).reshape(-1)
        in_maps.append({
            "ls": ls_c.T.copy(), "ef": ef_c.T.copy(),
            "qrm": q_sorted[row_idx].copy(),
            "rt": rt_sorted[row_idx].T.copy(),
            "padT": pad_cnt[row_idx].reshape(1, B * 128).astype(BF),
            "wkv": np.concatenate([Wk, Wv], 1).astype(BF),
            "wee": np.concatenate([We, We], 1).astype(BF),
            "wsk": np.concatenate([Wskip, (Wskip @ w_vec)[:, None]], 1).astype(BF),
            "w1a": w1a_s, "w1b": W1[D:2 * D, :].astype(BF), "w2": W2.astype(BF),
            "identB": identB, "identF": identF,
            "ones1": np.ones((1, 128), BF),
            "neg4": np.full((1, 4), -1.0, BF),
            "eps4": np.full((1, 4), 1e-16, BF),
            "bskr": bskr, "bkvr": bkvr,
            "uT": np.tile(u_vec.reshape(1, D), (128, 1)),
            "b1c": b1c, "b2c": b2.reshape(128, 1).astype(np.float32),
        })

    trace = bool(os.environ.get("K_TRACE"))
    res = run_bass_kernel_spmd(nc, in_maps, list(range(N_CORES)), trace=trace,
                               tmpdir=os.environ.get("K_TRACE_DIR") or None)
    global LAST_RESULT
    LAST_RESULT = res

    out_full = np.empty((NR, D), np.float32)
    for core in range(N_CORES):
        oc = np.asarray(res.results[core]["out"], dtype=np.float32)  # [128, B*128]
        row_idx = (128 * (N_CORES * np.arange(B)[:, None] + core) +
                   np.arange(128)[None, :]).reshape(-1)
        valid = row_idx < NR
        out_full[order[row_idx[valid]]] = oc.T[valid]
    return out_full
